# revision 1
# baseline (speedup 1.0000x reference)
"""Trainium2 Bass kernel for GQA causal sliding-window self-attention.

Sharding: 8 cores = 2 (batch) x 4 (KV-head groups). Each core handles one
batch element and one KV head with its 3 GQA query heads. The output
projection is computed per-group against the matching Wproj column slice;
the 4 partial outputs per batch are summed on the host.

Everything on-chip runs in feature-major ("transposed") layout so that all
matmul contractions have their contraction dim on SBUF partitions and all
DRAM traffic is contiguous. fp32r matmuls (full-rate) with fp32 PSUM
accumulation. Sliding-window/causal masking is applied by accumulating a
host-precomputed -1e9 additive mask tile into the scores PSUM via an
identity matmul (exp then underflows to exactly 0).
"""

import os
import sys
import numpy as np

sys.path.insert(0, "/opt/trn_rl_repo")

from contextlib import ExitStack

from concourse import mybir, bacc, tile
from concourse.bass_utils import run_bass_kernel_spmd

f32 = mybir.dt.float32
f32r = mybir.dt.float32r
AF = mybir.ActivationFunctionType

B, T, C = 2, 2048, 1536
H, KV, D = 12, 4, 128
REP = H // KV          # 3 query heads per kv head
QD = REP * D           # 384
VE_GATE_CH = 12
N_CORES = 8
TC = 512               # t-chunk width (matmul moving free dim)
NTC = T // TC          # 4
NCC = C // 128         # 12 contraction chunks
NST = T // 128         # 16 s-tiles

_EPS = float(np.finfo(np.float32).eps)
# all scale constants folded into the q-side rsqrt:
#   rq = (1.2*1.2/sqrt(D)) * rsqrt(mean(q^2)+eps),  rk = rsqrt(mean(k^2)+eps)
_LNCQ = float(np.log(1.2 * 1.2 / np.sqrt(D)))
_MASKVAL = -1.0e9

_CACHE = {}


def _setup_act_tables():
    """Reorder activation-table sets so ln+exp share one set (avoids ~33
    table reloads).  Patches both the bacc-side set picker and the walrus
    --act-root-json (they must agree on set indices)."""
    try:
        import json
        import tempfile
        import concourse.hw_specs as hw_specs
        import concourse.bacc as bacc_mod
        from neuronxcc.driver.Job import Job
        from neuronxcc.driver.jobs.support.FindActInfo import findActInfoFile

        src = findActInfoFile(Job.getPackageDir(), "gen3")
        if not src or not os.path.exists(src):
            return
        src_dir = os.path.dirname(src)
        dst = os.path.join(tempfile.gettempdir(), "bass_act_pwp_lnexp")
        os.makedirs(dst, exist_ok=True)
        for f in os.listdir(src_dir):
            tgt = os.path.join(dst, f)
            if not os.path.exists(tgt):
                try:
                    os.symlink(os.path.join(src_dir, f), tgt)
                except OSError:
                    pass
        d = json.load(open(src))
        sets = d["act_func_sets"]
        idx = [i for i, s in enumerate(sets)
               if s["name"] == "natural_log_exp_and_others"]
        if not idx:
            return
        sets.insert(0, sets.pop(idx[0]))
        jp = os.path.join(dst, "act_info.json")
        if os.path.lexists(jp):
            os.remove(jp)
        json.dump(d, open(jp, "w"))
        os.environ["BASS_ACT_ROOT_JSON_PATH"] = jp

        orig = hw_specs.get_activation_tables

        def reordered(arch):
            t = orig(arch)
            key = "natural_log_exp_and_others"
            if key in t:
                out = {key: t[key]}
                out.update((k, v) for k, v in t.items() if k != key)
                return out
            return t

        hw_specs.get_activation_tables = reordered
        bacc_mod.get_activation_tables = reordered
    except Exception:
        pass


_setup_act_tables()


def _partial_deltas(window, win_finite):
    """Tile-offset classes (delta = t0 - s0) that need an additive mask."""
    deltas = set()
    for dlt in range(-(TC - 128), 0 + 1, 128):        # causal partials
        deltas.add(dlt)
    if win_finite:
        dlt = window - (window % 128)                  # window partials
        while dlt + (TC - 1) > window:
            if dlt >= -(TC - 128):
                deltas.add(dlt)
            dlt -= 128
    return sorted(deltas)


def _build(window: int):
    win_finite = 0 <= window < T
    deltas = _partial_deltas(window, win_finite)
    wdeltas = [d for d in deltas if win_finite and d > window - (TC - 1)]
    wmin = min(wdeltas) if wdeltas else 0
    WIDE = TC + (TC - 128)                  # covers 4 deltas of 128
    NM = 2 if wdeltas else 1

    nc = bacc.Bacc("TRN2", target_bir_lowering=False, debug=False,
                   num_devices=N_CORES)

    xT = nc.dram_tensor("xT", [C, T], f32r, kind="ExternalInput")
    wqT = nc.dram_tensor("wqT", [C, QD], f32r, kind="ExternalInput")
    wkT = nc.dram_tensor("wkT", [C, D], f32r, kind="ExternalInput")
    wvT = nc.dram_tensor("wvT", [C, D], f32r, kind="ExternalInput")
    wpT = nc.dram_tensor("wpT", [QD, C], f32r, kind="ExternalInput")
    wg = nc.dram_tensor("wg", [VE_GATE_CH, 1], f32r, kind="ExternalInput")
    veT = nc.dram_tensor("veT", [D, T], f32r, kind="ExternalInput")
    cos2 = nc.dram_tensor("cos2", [128, T], f32r, kind="ExternalInput")
    sin2m = nc.dram_tensor("sin2m", [128, T], f32r, kind="ExternalInput")
    eye = nc.dram_tensor("eye", [128, 128], f32r, kind="ExternalInput")
    onesI = nc.dram_tensor("onesI", [128, 1], f32r, kind="ExternalInput")
    masksI = nc.dram_tensor("masksI", [NM * 128, WIDE], f32r, kind="ExternalInput")
    outT = nc.dram_tensor("outT", [C, T], f32, kind="ExternalOutput")

    with tile.TileContext(nc) as tc, ExitStack() as ctx:
        # ---- persistent SBUF pools ----
        pw = ctx.enter_context(tc.tile_pool(name="pw", bufs=1))
        pbig = ctx.enter_context(tc.tile_pool(name="pbig", bufs=1))
        prow = ctx.enter_context(tc.tile_pool(name="prow", bufs=6))
        pbc = ctx.enter_context(tc.tile_pool(name="pbc", bufs=4))

        # ---- PSUM pools (8 banks total, elastic shared tags) ----
        psAO = ctx.enter_context(tc.tile_pool(name="psAO", bufs=1, space="PSUM"))
        psR = ctx.enter_context(tc.tile_pool(name="psR", bufs=2, space="PSUM"))
        psSY = ctx.enter_context(tc.tile_pool(name="psSY", bufs=5, space="PSUM"))

        # small constants (needed from phase 1)
        wg_sb = pw.tile([VE_GATE_CH, 1], f32r, tag="wg")
        nc.sync.dma_start(wg_sb[:], wg.ap()[:])
        ones_sb = pw.tile([128, 1], f32r, tag="ones")
        nc.sync.dma_start(ones_sb[:], onesI.ap()[:])
        eye_sb = pw.tile([128, 128], f32r, tag="eye")
        nc.sync.dma_start(eye_sb[:], eye.ap()[:])
        masks_sb = pw.tile([128, NM, WIDE], f32r, tag="masks")
        eps_row = pw.tile([128, 1], f32, tag="epsr")
        nc.vector.memset(eps_row[:], _EPS)
        lncq_row = pw.tile([128, 1], f32, tag="lncq")
        nc.vector.memset(lncq_row[:], _LNCQ)

        # big persistent activations
        qT_sb = [pbig.tile([128, T], f32r, tag=f"qT{m}", name=f"qT{m}")
                 for m in range(REP)]
        kT_sb = pbig.tile([128, T], f32r, tag="kT")
        V_sb = pbig.tile([128, NST, D], f32r, tag="V")

        xT_re = xT.ap().rearrange("(cc p) t -> p cc t", p=128)

        # =========== phase 1: projections + gate + rope + rmsnorm ===========
        pending_pe = []   # deferred PE ops (sumsq matmuls, v transposes)

        def flush_pe(n=None):
            k = len(pending_pe) if n is None else min(n, len(pending_pe))
            for _ in range(k):
                pending_pe.pop(0)()

        with ExitStack() as ctx1:
            p1w = ctx1.enter_context(tc.tile_pool(name="p1w", bufs=1))
            pxt = ctx1.enter_context(tc.tile_pool(name="pxt", bufs=5))
            pcs = ctx1.enter_context(tc.tile_pool(name="pcs", bufs=2))
            ptmp = ctx1.enter_context(tc.tile_pool(name="ptmp", bufs=12))

            wk_sb = p1w.tile([128, NCC, D], f32r, tag="wk")
            wkT_re = wkT.ap().rearrange("(cc p) m -> p cc m", p=128)
            for g0 in range(0, NCC, 6):
                nc.sync.dma_start(wk_sb[:, g0:g0 + 6, :], wkT_re[:, g0:g0 + 6, :])
            wv_sb = p1w.tile([128, NCC, D], f32r, tag="wv")
            wvT_re = wvT.ap().rearrange("(cc p) m -> p cc m", p=128)

            HTC = TC // 2

            def load_xt(tci):
                eng = nc.sync
                halves = []
                for hh in range(2):
                    t0 = tci * TC + hh * HTC
                    xth = pxt.tile([128, NCC, HTC], f32r, tag="xt", name="xth")
                    for g0 in range(0, NCC, 4):
                        eng.dma_start(xth[:, g0:g0 + 4, :],
                                      xT_re[:, g0:g0 + 4, t0:t0 + HTC])
                    halves.append(xth)
                return halves

            xt_next = load_xt(0)
            for g0 in range(0, NCC, 6):
                nc.scalar.dma_start(wv_sb[:, g0:g0 + 6, :], wvT_re[:, g0:g0 + 6, :])
            wq_sb = p1w.tile([128, NCC, QD], f32r, tag="wq")
            wqT_re = wqT.ap().rearrange("(cc p) m -> p cc m", p=128)
            for g0 in range(0, NCC, 3):
                nc.sync.dma_start(wq_sb[:, g0:g0 + 3, :], wqT_re[:, g0:g0 + 3, :])

            for tci in range(NTC):
                t0 = tci * TC
                xt = xt_next
                cs = pcs.tile([128, TC], f32r, tag="cs")
                nc.sync.dma_start(cs[:], cos2.ap()[:, t0:t0 + TC])
                sn = pcs.tile([128, TC], f32r, tag="sn")
                nc.sync.dma_start(sn[:], sin2m.ap()[:, t0:t0 + TC])
                ve_t = pcs.tile([128, TC], f32r, tag="vet")
                nc.sync.dma_start(ve_t[:], veT.ap()[:, t0:t0 + TC])
                if tci + 1 < NTC:
                    xt_next = load_xt(tci + 1)

                # ve gate: sigmoid(x[:, :12] @ wg); the *3 is folded into veT
                zg = psR.tile([1, TC], f32, tag="row")
                nc.tensor.matmul(zg[0:1, 0:HTC], wg_sb[:],
                                 xt[0][0:VE_GATE_CH, 0, :],
                                 start=True, stop=False)
                nc.tensor.matmul(zg[0:1, HTC:TC], wg_sb[:],
                                 xt[1][0:VE_GATE_CH, 0, :],
                                 start=False, stop=True)
                ez = prow.tile([1, TC], f32, tag="g")
                nc.scalar.activation(ez[:], zg[:], AF.Exp, scale=-1.0)
                ez1 = prow.tile([1, TC], f32, tag="g")
                nc.vector.tensor_scalar_add(ez1[:], ez[:], 1.0)
                grow = prow.tile([1, TC], f32, tag="g")
                nc.vector.reciprocal(grow[:], ez1[:])
                gbc = pbc.tile([128, TC], f32, tag="bc")
                nc.gpsimd.partition_broadcast(gbc[:], grow[:])

                streams = [("k", 0)] + [("q", m) for m in range(REP)] + [("v", 0)]
                for kind, m in streams:
                    acc = psSY.tile([128, TC], f32, tag="sy", name="acc")
                    for hh in range(2):
                        for cc in range(NCC):
                            if kind == "q":
                                lhsT = wq_sb[:, cc, m * D:(m + 1) * D]
                            elif kind == "k":
                                lhsT = wk_sb[:, cc, :]
                            else:
                                lhsT = wv_sb[:, cc, :]
                            nc.tensor.matmul(
                                acc[:, hh * HTC:(hh + 1) * HTC], lhsT,
                                xt[hh][:, cc, :],
                                start=(cc == 0 and hh == 0),
                                stop=(cc == NCC - 1 and hh == 1))

                    if kind == "v":
                        # v += gate * ve; then transpose into natural [s, D]
                        vtmp = ptmp.tile([128, TC], f32, tag="t")
                        nc.vector.tensor_mul(vtmp[:], gbc[:], ve_t[:])
                        vfull = ptmp.tile([128, TC], f32r, tag="t")
                        nc.vector.tensor_add(vfull[:], vtmp[:], acc[:])

                        def vtrans(tci=tci, vfull=vfull):
                            for j in range(TC // 128):
                                st = tci * (TC // 128) + j
                                vtr = psSY.tile([128, 128], f32r, tag="sy",
                                                name="vtr")
                                nc.tensor.transpose(
                                    vtr[:], vfull[:, j * 128:(j + 1) * 128],
                                    eye_sb[:])
                                if j % 2 == 0:
                                    nc.scalar.copy(V_sb[:, st, :], vtr[:])
                                else:
                                    nc.vector.tensor_copy(V_sb[:, st, :], vtr[:])
                        pending_pe.append(vtrans)
                        continue

                    # q/k: evacuate PSUM early, then rmsnorm stats off SBUF
                    qraw = ptmp.tile([128, TC], f32r, tag="t")
                    nc.scalar.copy(qraw[:], acc[:])
                    sqr = ptmp.tile([128, TC], f32r, tag="t")
                    nc.scalar.activation(sqr[:], qraw[:], AF.Square)

                    def final(kind=kind, m=m, qraw=qraw, sqr=sqr, t0=t0,
                              cs=cs, sn=sn):
                        ss = psR.tile([1, TC], f32, tag="row", name="ss")
                        nc.tensor.matmul(ss[:], ones_sb[:], sqr[:],
                                         start=True, stop=True)
                        lnr = prow.tile([1, TC], f32, tag="r", name="lnr")
                        nc.scalar.activation(lnr[:], ss[:], AF.Ln,
                                             scale=1.0 / D, bias=eps_row[0:1, :])
                        rr = prow.tile([1, TC], f32, tag="r", name="rr")
                        if kind == "q":
                            nc.scalar.activation(rr[:], lnr[:], AF.Exp,
                                                 scale=-0.5,
                                                 bias=lncq_row[0:1, :])
                        else:
                            nc.scalar.activation(rr[:], lnr[:], AF.Exp,
                                                 scale=-0.5, bias=0.0)
                        rbc = pbc.tile([128, TC], f32, tag="bc", name="rbc")
                        nc.gpsimd.partition_broadcast(rbc[:], rr[:])

                        qn = ptmp.tile([128, TC], f32r, tag="t", name="qn")
                        nc.vector.tensor_mul(qn[:], rbc[:], qraw[:])
                        # rope: out = qn*[cos;cos] + swap(qn)*[sin;-sin]
                        qsw = ptmp.tile([128, TC], f32r, tag="t", name="qsw")
                        nc.sync.dma_start(qsw[0:64, :], qn[64:128, :])
                        nc.sync.dma_start(qsw[64:128, :], qn[0:64, :])
                        ta = ptmp.tile([128, TC], f32r, tag="t", name="ta")
                        nc.vector.tensor_mul(ta[:], qn[:], cs[:])
                        tb = ptmp.tile([128, TC], f32r, tag="t", name="tb")
                        nc.vector.tensor_mul(tb[:], qsw[:], sn[:])
                        dst = qT_sb[m] if kind == "q" else kT_sb
                        nc.vector.tensor_add(dst[:, t0:t0 + TC], ta[:], tb[:])
                    pending_pe.append(final)

                    # keep PE dense: flush one deferred op per stream
                    if len(pending_pe) > 1:
                        flush_pe(1)
                if tci == 1:
                    nc.scalar.dma_start(
                        masks_sb[:],
                        masksI.ap().rearrange("(nd p) t -> p nd t", p=128))
            flush_pe()

        # =========== phase 2+3 per t-chunk: attention + out-proj ===========
        pw2 = ctx.enter_context(tc.tile_pool(name="pw2", bufs=1))
        wp_sb = pw2.tile([128, REP, C], f32r, tag="wp")
        nc.scalar.dma_start(wp_sb[:], wpT.ap().rearrange("(qc p) c -> p qc c",
                                                         p=128))
        yT_sb = [pw2.tile([128, T], f32r, tag=f"yT{m}", name=f"yT{m}")
                 for m in range(REP)]
        pP = ctx.enter_context(tc.tile_pool(name="pP", bufs=6))
        pout = ctx.enter_context(tc.tile_pool(name="pout", bufs=3))

        for tci in range(NTC):
            t0 = tci * TC
            if win_finite:
                st_min = max(0, (t0 - window - 127) // 128 + 1)
            else:
                st_min = 0
            st_max = (t0 + TC - 1) // 128
            sts = list(range(st_min, st_max + 1))

            for h in range(REP):
                yU = psSY.tile([128, TC], f32, tag="sy", name="yU")
                den = psR.tile([1, TC], f32, tag="row", name="den")
                q_rhs = qT_sb[h][:, t0:t0 + TC]
                pends = []    # software-pipeline den/Y two s-tiles behind
                for idx, st in enumerate(sts):
                    s0 = st * 128
                    delta = t0 - s0
                    causal_p = delta <= 0
                    window_p = win_finite and delta > window - (TC - 1)
                    nmm = int(causal_p) + int(window_p)
                    # valid column range for this s-tile (outside it every
                    # element is masked, so P is exactly 0 there and the
                    # mask-MM covers those columns of the scores bank)
                    v0 = max(0, -delta) if causal_p else 0
                    v1 = min(TC, window - delta + 128) if window_p else TC
                    if v1 - v0 < 256:      # stay on the fp32r fast path
                        v0, v1 = 0, TC
                    sc = psSY.tile([128, TC], f32, tag="sy", name="sc")
                    nc.tensor.matmul(sc[:, v0:v1], kT_sb[:, s0:s0 + 128],
                                     qT_sb[h][:, t0 + v0:t0 + v1],
                                     start=True, stop=(nmm == 0))
                    if causal_p:    # masked cols [0, 128-delta)
                        c0, c1 = 0, max(256, min(TC, 128 - delta))
                        off = delta + (TC - 128)
                        nmm -= 1
                        nc.tensor.matmul(sc[:, c0:c1], eye_sb[:],
                                         masks_sb[:, 0, off + c0:off + c1],
                                         start=False, stop=(nmm == 0))
                    if window_p:    # masked cols suffix
                        c0 = min(TC - 256,
                                 (window - delta + 1) // 128 * 128)
                        c1 = TC
                        off = delta - wmin
                        nmm -= 1
                        nc.tensor.matmul(sc[:, c0:c1], eye_sb[:],
                                         masks_sb[:, 1, off + c0:off + c1],
                                         start=False, stop=(nmm == 0))
                    if len(pends) >= 2:
                        pends.pop(0)()
                    P = pP.tile([128, TC], f32r, tag="P", name="P")
                    nc.scalar.activation(P[:, v0:v1], sc[:, v0:v1], AF.Exp)

                    def mk(idx=idx, st=st, P=P, v0=v0, v1=v1):
                        first, last = idx == 0, idx == len(sts) - 1
                        def go():
                            nc.tensor.matmul(den[0:1, v0:v1], ones_sb[:],
                                             P[:, v0:v1],
                                             start=first, stop=last)
                            nc.tensor.matmul(yU[:, v0:v1], V_sb[:, st, :],
                                             P[:, v0:v1],
                                             start=first, stop=last)
                        return go
                    pends.append(mk())
                while pends:
                    pends.pop(0)()
                dinv = prow.tile([1, TC], f32, tag="r", name="dinv")
                nc.vector.reciprocal(dinv[:], den[:])
                dbc = pbc.tile([128, TC], f32, tag="bc", name="dbc")
                nc.gpsimd.partition_broadcast(dbc[:], dinv[:])
                nc.vector.tensor_mul(yT_sb[h][:, t0:t0 + TC], dbc[:], yU[:])

            # ---- out-proj for this t-chunk ----
            for cc in range(NCC):
                if tci == NTC - 1 and cc % 2 == 1:
                    o = psSY.tile([128, TC], f32, tag="sy", name="o")
                else:
                    o = psAO.tile([128, TC], f32, tag="ao", name="o")
                for m in range(REP):
                    nc.tensor.matmul(o[:], wp_sb[:, m, cc * 128:(cc + 1) * 128],
                                     yT_sb[m][:, t0:t0 + TC],
                                     start=(m == 0), stop=(m == REP - 1))
                ot = pout.tile([128, TC], f32, tag="ot", name="ot")
                if cc % 2 == 0:
                    nc.vector.tensor_copy(ot[:], o[:])
                else:
                    nc.scalar.copy(ot[:], o[:])
                nc.sync.dma_start(outT.ap()[cc * 128:(cc + 1) * 128,
                                            t0:t0 + TC], ot[:])

    nc.compile()
    nc._mask_cfg = {"wide": WIDE, "cmin": -(TC - 128), "wmin": wmin}
    return nc


def _prep_inputs(nc, window, x, ve, cos, sin, Wq, Wk, Wv, Wproj, Wg):
    """Build the 8 per-core input maps (host-side sharding + transposes)."""
    win_finite = 0 <= window < T
    cosT = np.ascontiguousarray(cos.reshape(T, D // 2).T)
    sinT = np.ascontiguousarray(sin.reshape(T, D // 2).T)
    cos2 = np.concatenate([cosT, cosT], axis=0)
    sin2m = np.concatenate([sinT, -sinT], axis=0)
    eye = np.eye(128, dtype=np.float32)
    ones = np.ones((128, 1), dtype=np.float32)

    ds = np.arange(128)[:, None]
    wcfg = nc._mask_cfg
    j = np.arange(wcfg["wide"])[None, :]
    mc = np.where(j + wcfg["cmin"] - ds >= 0, 0.0, _MASKVAL).astype(np.float32)
    rows = [mc]
    if win_finite:
        mw = np.where(j + wcfg["wmin"] - ds <= window, 0.0,
                      _MASKVAL).astype(np.float32)
        rows.append(mw)
    masks = np.concatenate(rows, axis=0)

    xTb = [np.ascontiguousarray(x[b].T) for b in range(B)]

    in_maps = []
    for core in range(N_CORES):
        b, g = divmod(core, KV)
        sl_q = slice(g * QD, (g + 1) * QD)
        sl_d = slice(g * D, (g + 1) * D)
        in_maps.append({
            "xT": xTb[b],
            "wqT": np.ascontiguousarray(Wq[sl_q].T),
            "wkT": np.ascontiguousarray(Wk[sl_d].T),
            "wvT": np.ascontiguousarray(Wv[sl_d].T),
            "wpT": np.ascontiguousarray(Wproj[:, sl_q].T),
            "wg": np.ascontiguousarray(Wg[g].reshape(VE_GATE_CH, 1)),
            "veT": np.ascontiguousarray(3.0 * ve[b, :, sl_d].T),
            "cos2": cos2, "sin2m": sin2m, "eye": eye, "onesI": ones,
            "masksI": masks,
        })
    return in_maps


def kernel(x, ve, cos, sin, Wq, Wk, Wv, Wproj, Wg, window, _trace=False):
    window = int(window)
    if window not in _CACHE:
        _CACHE[window] = _build(window)
    nc = _CACHE[window]

    in_maps = _prep_inputs(nc, window,
                           np.asarray(x, np.float32), np.asarray(ve, np.float32),
                           np.asarray(cos, np.float32), np.asarray(sin, np.float32),
                           np.asarray(Wq, np.float32), np.asarray(Wk, np.float32),
                           np.asarray(Wv, np.float32), np.asarray(Wproj, np.float32),
                           np.asarray(Wg, np.float32))

    res = run_bass_kernel_spmd(nc, in_maps, core_ids=list(range(N_CORES)),
                               trace=_trace)

    out = np.empty((B, T, C), dtype=np.float32)
    for b in range(B):
        acc = res.results[b * KV]["outT"].copy()
        for g in range(1, KV):
            acc += res.results[b * KV + g]["outT"]
        out[b] = acc.T
    if _trace:
        kernel._last_trace = res
    return out



# revision 21
# speedup vs baseline: 1.0010x; 1.0010x over previous
"""Trainium2 Bass kernel for GQA causal sliding-window self-attention.

Sharding: 8 cores = 2 (batch) x 4 (KV-head groups); host sums the 4
out-projection partials per batch.

v2 design:
- QKV projections in fp8 DoubleRow (x = xh+xl e4m3, W*64 = Wh+Wl e4m3,
  3-term expansion Wh*xh + Wh*xl + Wl*xh => 0.75 cols/unit vs 1.0 bf16).
- Scores/masks/projection in bf16 (1 cycle/row at any width -> exact
  valid-band trimming, no 256-col padding).
- Attention P = exp(scores) kept in bf16; AV matmul runs in the flipped
  orientation out[t, d] = P[s,t]^T V[s,d] with a 129th V column of 64.0
  so the softmax denominator rides in the same matmul. The divide is a
  per-partition tensor_scalar_mul; y tiles transpose back via PE.
- RMSNorm stats for the 4 q/k streams of a t-chunk are packed into one
  [4, TC] psum row-block (per-stream scale folded into the sum-of-squares
  ones-vector, per-row bias AP), so ln/exp run once per chunk.
- Output projection partials are evacuated to fp16 and summed on host.
"""

import os
import sys
import numpy as np
import ml_dtypes

sys.path.insert(0, "/opt/trn_rl_repo")

from contextlib import ExitStack

from concourse import mybir, bacc, tile
from concourse.bass_utils import run_bass_kernel_spmd

f32 = mybir.dt.float32
bf16 = mybir.dt.bfloat16
f16 = mybir.dt.float16
e4 = mybir.dt.float8e4
AF = mybir.ActivationFunctionType
DRM = mybir.MatmulPerfMode.DoubleRow

E4 = ml_dtypes.float8_e4m3
BF = ml_dtypes.bfloat16

B, T, C = 2, 2048, 1536
H, KV, D = 12, 4, 128
REP = H // KV          # 3 query heads per kv head
QD = REP * D           # 384
VE_GATE_CH = 12
N_CORES = 8
TC = 512               # t-chunk width
NTC = T // TC          # 4
NCC = C // 128         # 12 contraction chunks
NST = T // 128         # 16 s-tiles
VW = 130               # V_sb row pitch; col 128 holds the den constant

WSC = 64.0             # weight prescale before fp8 split
DEN = 64.0             # V den-column value: y = yU / (WSC * sum P)

_EPS = float(np.finfo(np.float32).eps)
_CQ = 1.2 * 1.2 / np.sqrt(D)       # all q-side scale constants
_RMSB = (float((WSC / 1.0) ** 2 * _EPS),       # k-stream ln bias
         float((WSC / _CQ) ** 2 * _EPS))      # q-stream ln bias
_MASKVAL = -1000.0

_CACHE = {}


def _setup_act_tables():
    """Reorder activation-table sets so ln+exp share one set."""
    try:
        import json
        import tempfile
        import concourse.hw_specs as hw_specs
        import concourse.bacc as bacc_mod
        from neuronxcc.driver.Job import Job
        from neuronxcc.driver.jobs.support.FindActInfo import findActInfoFile

        src = findActInfoFile(Job.getPackageDir(), "gen3")
        if not src or not os.path.exists(src):
            return
        src_dir = os.path.dirname(src)
        dst = os.path.join(tempfile.gettempdir(), "bass_act_pwp_lnexp")
        os.makedirs(dst, exist_ok=True)
        for f in os.listdir(src_dir):
            tgt = os.path.join(dst, f)
            if not os.path.exists(tgt):
                try:
                    os.symlink(os.path.join(src_dir, f), tgt)
                except OSError:
                    pass
        d = json.load(open(src))
        sets = d["act_func_sets"]
        idx = [i for i, s in enumerate(sets)
               if s["name"] == "natural_log_exp_and_others"]
        if not idx:
            return
        sets.insert(0, sets.pop(idx[0]))
        jp = os.path.join(dst, "act_info.json")
        if os.path.lexists(jp):
            os.remove(jp)
        json.dump(d, open(jp, "w"))
        os.environ["BASS_ACT_ROOT_JSON_PATH"] = jp

        orig = hw_specs.get_activation_tables

        def reordered(arch):
            t = orig(arch)
            key = "natural_log_exp_and_others"
            if key in t:
                out = {key: t[key]}
                out.update((k, v) for k, v in t.items() if k != key)
                return out
            return t

        hw_specs.get_activation_tables = reordered
        bacc_mod.get_activation_tables = reordered
    except Exception:
        pass


_setup_act_tables()


def _build(window: int):
    win_finite = 0 <= window < T
    # additive-mask tile classes, indexed by delta = t0 - s0
    cmin = -(TC - 128)
    wdeltas = []
    if win_finite:
        dlt = window - (window % 128)
        while dlt + (TC - 1) > window:
            if dlt >= cmin:
                wdeltas.append(dlt)
            dlt -= 128
    wmin = min(wdeltas) if wdeltas else 0
    WIDE = TC + (TC - 128)
    NM = 2 if wdeltas else 1

    nc = bacc.Bacc("TRN2", target_bir_lowering=False, debug=False,
                   num_devices=N_CORES)

    xA = nc.dram_tensor("xA", [128, NCC, 2, T], e4, kind="ExternalInput")
    xB = nc.dram_tensor("xB", [128, NCC, T], e4, kind="ExternalInput")
    x12 = nc.dram_tensor("x12", [VE_GATE_CH, T], bf16, kind="ExternalInput")
    wqA = nc.dram_tensor("wqA", [128, NCC, 2, QD], e4, kind="ExternalInput")
    wqB = nc.dram_tensor("wqB", [128, NCC, QD], e4, kind="ExternalInput")
    wkA = nc.dram_tensor("wkA", [128, NCC, 2, D], e4, kind="ExternalInput")
    wkB = nc.dram_tensor("wkB", [128, NCC, D], e4, kind="ExternalInput")
    wvA = nc.dram_tensor("wvA", [128, NCC, 2, D], e4, kind="ExternalInput")
    wvB = nc.dram_tensor("wvB", [128, NCC, D], e4, kind="ExternalInput")
    wpT = nc.dram_tensor("wpT", [128, REP, C], bf16, kind="ExternalInput")
    wg = nc.dram_tensor("wg", [VE_GATE_CH, 1], bf16, kind="ExternalInput")
    veT = nc.dram_tensor("veT", [128, T], bf16, kind="ExternalInput")
    cos2 = nc.dram_tensor("cos2", [128, T], bf16, kind="ExternalInput")
    sin2m = nc.dram_tensor("sin2m", [128, T], bf16, kind="ExternalInput")
    eye = nc.dram_tensor("eye", [128, 128], bf16, kind="ExternalInput")
    onesA = nc.dram_tensor("onesA", [128, 4], bf16, kind="ExternalInput")
    masksI = nc.dram_tensor("masksI", [128, NM, WIDE], bf16,
                            kind="ExternalInput")
    outT = nc.dram_tensor("outT", [C, T], f16, kind="ExternalOutput")

    with tile.TileContext(nc) as tc, ExitStack() as ctx:
        pw = ctx.enter_context(tc.tile_pool(name="pw", bufs=1))
        pbig = ctx.enter_context(tc.tile_pool(name="pbig", bufs=1))
        prow = ctx.enter_context(tc.tile_pool(name="prow", bufs=4))
        pbc = ctx.enter_context(tc.tile_pool(name="pbc", bufs=4))
        pxs = ctx.enter_context(tc.tile_pool(name="pxs", bufs=2))
        pcs = ctx.enter_context(tc.tile_pool(name="pcs", bufs=2))
        ptmp = ctx.enter_context(tc.tile_pool(name="ptmp", bufs=10))
        pP = ctx.enter_context(tc.tile_pool(name="pP", bufs=16))
        pout = ctx.enter_context(tc.tile_pool(name="pout", bufs=3))

        # PSUM pools
        psAO = ctx.enter_context(tc.tile_pool(name="psAO", bufs=2,
                                              space="PSUM"))
        psS = ctx.enter_context(tc.tile_pool(name="psS", bufs=2, space="PSUM"))
        psR = ctx.enter_context(tc.tile_pool(name="psR", bufs=2, space="PSUM"))
        psY = ctx.enter_context(tc.tile_pool(name="psY", bufs=2, space="PSUM"))

        # ---- constants ----
        wg_sb = pw.tile([VE_GATE_CH, 1], bf16, tag="wg")
        nc.sync.dma_start(wg_sb[:], wg.ap()[:])
        eye_sb = pw.tile([128, 128], bf16, tag="eye")
        nc.sync.dma_start(eye_sb[:], eye.ap()[:])
        onesA_sb = pw.tile([128, 4], bf16, tag="onesA")
        nc.sync.dma_start(onesA_sb[:], onesA.ap()[:])
        rbias_k = pw.tile([1, 1], f32, tag="rbk")
        nc.vector.memset(rbias_k[:], _RMSB[0])
        rbias_q = pw.tile([1, 1], f32, tag="rbq")
        nc.vector.memset(rbias_q[:], _RMSB[1])
        x12_sb = pw.tile([VE_GATE_CH, T], bf16, tag="x12")
        nc.sync.dma_start(x12_sb[:], x12.ap()[:])
        masks_sb = pw.tile([128, NM, WIDE], bf16, tag="masks")
        nc.sync.dma_start(masks_sb[:], masksI.ap()[:])

        # ---- persistent activations ----
        qT_sb = [pbig.tile([128, T], bf16, tag=f"qT{m}", name=f"qT{m}")
                 for m in range(REP)]
        kT_sb = pbig.tile([128, T], bf16, tag="kT")
        V_sb = pbig.tile([128, NST, VW], bf16, tag="V")
        nc.vector.memset(V_sb[:, :, 128:129], DEN)
        yT_sb = [pbig.tile([128, T], bf16, tag=f"yT{m}", name=f"yT{m}")
                 for m in range(REP)]
        wp_sb = pw.tile([128, REP, C], bf16, tag="wp")
        nc.scalar.dma_start(wp_sb[:], wpT.ap()[:])

        # ---- weight loads (fp8) ----
        wA_sb = {}
        wB_sb = {}
        for nm, dA, dB, M in (("q", wqA, wqB, QD), ("k", wkA, wkB, D),
                              ("v", wvA, wvB, D)):
            a = pw.tile([128, NCC, 2, M], e4, tag=f"w{nm}A")
            nc.sync.dma_start(a[:], dA.ap()[:])
            b = pw.tile([128, NCC, M], e4, tag=f"w{nm}B")
            nc.sync.dma_start(b[:], dB.ap()[:])
            wA_sb[nm] = a
            wB_sb[nm] = b

        def load_x(tci):
            t0 = tci * TC
            xa = pxs.tile([128, NCC, 2, TC], e4, tag="xa", name="xa")
            for g0 in range(0, NCC, 3):
                nc.sync.dma_start(xa[:, g0:g0 + 3, :, :],
                                  xA.ap()[:, g0:g0 + 3, :, t0:t0 + TC])
            xb = pxs.tile([128, NCC, TC], e4, tag="xb", name="xb")
            for g0 in range(0, NCC, 6):
                nc.sync.dma_start(xb[:, g0:g0 + 6, :],
                                  xB.ap()[:, g0:g0 + 6, t0:t0 + TC])
            return xa, xb

        # =================== phase 1 for one t-chunk ===================
        def phase1(tci, xs, xs_next):
            t0 = tci * TC
            xa, xb = xs
            cs = pcs.tile([128, TC], bf16, tag="cs", name="cs")
            nc.sync.dma_start(cs[:], cos2.ap()[:, t0:t0 + TC])
            sn = pcs.tile([128, TC], bf16, tag="sn", name="sn")
            nc.sync.dma_start(sn[:], sin2m.ap()[:, t0:t0 + TC])
            ve_t = pcs.tile([128, TC], bf16, tag="vet", name="vet")
            nc.sync.dma_start(ve_t[:], veT.ap()[:, t0:t0 + TC])

            # gate: z then sigmoid via exp/add/recip (stays in ln/exp table)
            zrow = psR.tile([1, TC], f32, tag="row", name="zrow")
            nc.tensor.matmul(zrow[:], wg_sb[:], x12_sb[:, t0:t0 + TC],
                             start=True, stop=True)
            ez = prow.tile([1, TC], f32, tag="g", name="ez")
            nc.scalar.activation(ez[:], zrow[:], AF.Exp, scale=-1.0)
            ez1 = prow.tile([1, TC], f32, tag="g", name="ez1")
            nc.vector.tensor_scalar_add(ez1[:], ez[:], 1.0)
            grow = prow.tile([1, TC], f32, tag="g", name="grow")
            nc.vector.reciprocal(grow[:], ez1[:])
            growb = prow.tile([1, TC], bf16, tag="g", name="growb")
            nc.vector.tensor_copy(growb[:], grow[:])
            gbc = pbc.tile([128, TC], bf16, tag="bc", name="gbc")
            nc.gpsimd.partition_broadcast(gbc[:], growb[:])

            streams = [("k", 0), ("q", 0), ("q", 1), ("q", 2), ("v", 0)]
            qraws = {}
            rrs = {}

            for si, (kind, m) in enumerate(streams):
                acc = psAO.tile([128, TC], f32, tag="ao", name="acc")
                wA = wA_sb[kind]
                wB = wB_sb[kind]
                if kind == "q":
                    mA = slice(m * D, (m + 1) * D)
                else:
                    mA = slice(0, D)
                for cc in range(0, NCC, 2):
                    nc.tensor.matmul(acc[:], wA[:, cc, :, mA],
                                     xa[:, cc, :, :],
                                     start=(cc == 0), stop=False,
                                     perf_mode=DRM)
                    nc.tensor.matmul(acc[:], wA[:, cc + 1, :, mA],
                                     xa[:, cc + 1, :, :],
                                     start=False, stop=False, perf_mode=DRM)
                    nc.tensor.matmul(acc[:], wB[:, cc:cc + 2, mA],
                                     xb[:, cc:cc + 2, :],
                                     start=False, stop=(cc == NCC - 2),
                                     perf_mode=DRM)

                if kind == "v":
                    vee = ptmp.tile([128, TC], bf16, tag="t", name="vee")
                    nc.vector.tensor_mul(vee[:], gbc[:], ve_t[:])
                    vfull = ptmp.tile([128, TC], bf16, tag="t", name="vfull")
                    nc.vector.tensor_add(vfull[:], acc[:], vee[:])
                    for j in range(TC // 128):
                        st = tci * (TC // 128) + j
                        vtr = psS.tile([128, 128], bf16, tag="sc",
                                       name="vtr")
                        nc.tensor.transpose(vtr[:],
                                            vfull[:, j * 128:(j + 1) * 128],
                                            eye_sb[:])
                        nc.vector.tensor_copy(V_sb[:, st, 0:128], vtr[:])
                    continue

                r = si  # 0..3 = (k, q0, q1, q2)
                qraw = ptmp.tile([128, TC], bf16, tag="t", name="qraw")
                nc.scalar.copy(qraw[:], acc[:])
                qraws[(kind, m)] = qraw
                sqr = ptmp.tile([128, TC], bf16, tag="t", name="sqr")
                nc.vector.tensor_mul(sqr[:], qraw[:], qraw[:])
                ss = psR.tile([1, TC], f32, tag="row", name="ss")
                nc.tensor.matmul(ss[:], onesA_sb[:, r:r + 1],
                                 sqr[:], start=True, stop=True)
                # rr = exp(-0.5 * ln(A*ss + B)) = c/WSC * rsqrt(mean+eps)
                lnr = prow.tile([1, TC], f32, tag="g", name="lnr")
                nc.scalar.activation(lnr[:], ss[:], AF.Ln,
                                     bias=(rbias_k if kind == "k"
                                           else rbias_q)[0:1, :])
                rr = prow.tile([1, TC], bf16, tag="g", name="rr")
                nc.scalar.activation(rr[:], lnr[:], AF.Exp, scale=-0.5)
                rrs[(kind, m)] = rr

            for si, (kind, m) in enumerate(streams[:4]):
                rbc = pbc.tile([128, TC], bf16, tag="bc", name="rbc")
                nc.gpsimd.partition_broadcast(rbc[:], rrs[(kind, m)][:])
                qraw = qraws[(kind, m)]
                qn = ptmp.tile([128, TC], bf16, tag="t", name="qn")
                nc.vector.tensor_mul(qn[:], rbc[:], qraw[:])
                qsw = ptmp.tile([128, TC], bf16, tag="t", name="qsw")
                nc.sync.dma_start(qsw[0:64, :], qn[64:128, :])
                nc.sync.dma_start(qsw[64:128, :], qn[0:64, :])
                ta = ptmp.tile([128, TC], bf16, tag="t", name="ta")
                nc.vector.tensor_mul(ta[:], qn[:], cs[:])
                tb = ptmp.tile([128, TC], bf16, tag="t", name="tb")
                nc.vector.tensor_mul(tb[:], qsw[:], sn[:])
                dst = qT_sb[m] if kind == "q" else kT_sb
                nc.vector.tensor_add(dst[:, t0:t0 + TC], ta[:], tb[:])

        # =================== phase 2+3 for one t-chunk ===================
        def phase23(tci):
            t0 = tci * TC
            if win_finite:
                st_min = max(0, (t0 - window - 127) // 128 + 1)
            else:
                st_min = 0
            st_max = (t0 + TC - 1) // 128
            sts = list(range(st_min, st_max + 1))

            for h in range(REP):
                Ps = {}
                for st in sts:
                    s0 = st * 128
                    delta = t0 - s0
                    causal_p = delta <= 0
                    window_p = win_finite and delta > window - (TC - 1)
                    nmm = int(causal_p) + int(window_p)
                    v0 = max(0, -delta) if causal_p else 0
                    v1 = min(TC, window - delta + 128) if window_p else TC
                    sc = psS.tile([128, TC], f32, tag="sc", name="sc")
                    nc.tensor.matmul(sc[:, v0:v1], kT_sb[:, s0:s0 + 128],
                                     qT_sb[h][:, t0 + v0:t0 + v1],
                                     start=True, stop=(nmm == 0))
                    if causal_p:
                        c0, c1 = 0, min(TC, 128 - delta)
                        off = delta + (TC - 128)
                        nmm -= 1
                        nc.tensor.matmul(sc[:, c0:c1], eye_sb[:],
                                         masks_sb[:, 0, off + c0:off + c1],
                                         start=False, stop=(nmm == 0))
                    if window_p:
                        c0 = max(0, (window - delta + 1) // 128 * 128)
                        c1 = v1
                        off = delta - wmin
                        nmm -= 1
                        nc.tensor.matmul(sc[:, c0:c1], eye_sb[:],
                                         masks_sb[:, 1, off + c0:off + c1],
                                         start=False, stop=(nmm == 0))
                    P = pP.tile([128, TC], bf16, tag="P", name="P")
                    nc.scalar.activation(P[:, v0:v1], sc[:, v0:v1], AF.Exp)
                    Ps[st] = (P, v0, v1)

                for j in range(TC // 128):
                    tj0 = t0 + j * 128
                    yU = psY.tile([128, VW], f32, tag="yU", name="yU")
                    # s-tiles whose written P range covers this t sub-block
                    jc = j * 128
                    stl = [st for st in sts
                           if Ps[st][1] <= jc and jc + 128 <= Ps[st][2]]
                    for i, st in enumerate(stl):
                        P, v0, v1 = Ps[st]
                        nc.tensor.matmul(
                            yU[:, 0:129], P[:, jc:jc + 128],
                            V_sb[:, st, 0:129],
                            start=(i == 0), stop=(i == len(stl) - 1))
                    dinv = prow.tile([128, 1], f32, tag="dv", name="dinv")
                    nc.vector.reciprocal(dinv[:], yU[:, 128:129])
                    ybf = ptmp.tile([128, 128], bf16, tag="y", name="ybf")
                    nc.vector.tensor_scalar_mul(ybf[:], yU[:, 0:128],
                                                dinv[:, 0:1])
                    ytr = psS.tile([128, 128], bf16, tag="sc", name="ytr")
                    nc.tensor.transpose(ytr[:], ybf[:], eye_sb[:])
                    nc.vector.tensor_copy(yT_sb[h][:, tj0:tj0 + 128], ytr[:])

            # ---- out-projection for this t-chunk ----
            for cc in range(NCC):
                o = psAO.tile([128, TC], f32, tag="ao", name="o")
                for m in range(REP):
                    nc.tensor.matmul(o[:],
                                     wp_sb[:, m, cc * 128:(cc + 1) * 128],
                                     yT_sb[m][:, t0:t0 + TC],
                                     start=(m == 0), stop=(m == REP - 1))
                ot = pout.tile([128, TC], f16, tag="ot", name="ot")
                nc.scalar.copy(ot[:], o[:])
                nc.sync.dma_start(outT.ap()[cc * 128:(cc + 1) * 128,
                                            t0:t0 + TC], ot[:])

        # =================== schedule ===================
        xs = load_x(0)
        xs_next = load_x(1)
        phase1(0, xs, xs_next)
        xs, xs_next = xs_next, load_x(2)
        phase1(1, xs, xs_next)
        phase23(0)
        xs, xs_next = xs_next, load_x(3)
        phase1(2, xs, xs_next)
        phase23(1)
        xs = xs_next
        phase1(3, xs, None)
        phase23(2)
        phase23(3)

    nc.compile()
    nc._mask_cfg = {"wide": WIDE, "cmin": cmin, "wmin": wmin, "nm": NM}
    return nc


def _q8(a):
    return a.astype(E4)


def _prep_inputs(nc, window, x, ve, cos, sin, Wq, Wk, Wv, Wproj, Wg):
    """Host-side sharding, transposes and fp8/bf16 packing."""
    cosT = np.ascontiguousarray(cos.reshape(T, D // 2).T)
    sinT = np.ascontiguousarray(sin.reshape(T, D // 2).T)
    cos2 = np.concatenate([cosT, cosT], axis=0).astype(BF)
    sin2m = np.concatenate([sinT, -sinT], axis=0).astype(BF)
    eye = np.eye(128, dtype=np.float32).astype(BF)

    # rmsnorm constants: rows (k, q0, q1, q2)
    # psum rows hold sum((WSC*q)^2); want rr = c_r/WSC * rsqrt(mean q^2+eps)
    #   = exp(-0.5*ln(A_r*ss + B_r)), A_r = 1/(c_r^2 D WSC^2)*WSC^2 ... =>
    #   A_r = 1/(c_r^2 * D * WSC^2) applied to raw ss, B_r=(WSC/c_r)^2*eps
    cs_ = [1.0, _CQ, _CQ, _CQ]
    onesA = np.zeros((128, 4), np.float32)
    for r, c in enumerate(cs_):
        onesA[:, r] = 1.0 / (c * c * D)
    onesA = onesA.astype(BF)

    ds = np.arange(128)[:, None]
    wcfg = nc._mask_cfg
    j = np.arange(wcfg["wide"])[None, :]
    mc = np.where(j + wcfg["cmin"] - ds >= 0, 0.0, _MASKVAL).astype(np.float32)
    rows = [mc]
    if wcfg["nm"] == 2:
        mw = np.where(j + wcfg["wmin"] - ds <= window, 0.0,
                      _MASKVAL).astype(np.float32)
        rows.append(mw)
    masks = np.stack(rows, axis=1).astype(BF)  # [128, NM, WIDE]

    def pack_w(W):          # W [M, C] -> hi/lo interleaved + hi
        W64 = (W.T * WSC).astype(np.float32)          # [C, M]
        wh = _q8(W64)
        wl = _q8(W64 - wh.astype(np.float32))
        M = W.shape[0]
        a = np.empty((NCC, 128, 2, M), E4)
        a[:, :, 0, :] = wh.reshape(NCC, 128, M)
        a[:, :, 1, :] = wl.reshape(NCC, 128, M)
        b = np.ascontiguousarray(wh.reshape(NCC, 128, M).transpose(1, 0, 2))
        return np.ascontiguousarray(a.transpose(1, 0, 2, 3)), b

    xbatch = []
    for b in range(B):
        xT = x[b].T.astype(np.float32)                # [C, T]
        xh = _q8(xT)
        xl = _q8(xT - xh.astype(np.float32))
        a = np.empty((NCC, 128, 2, T), E4)
        a[:, :, 0, :] = xh.reshape(NCC, 128, T)
        a[:, :, 1, :] = xh.reshape(NCC, 128, T)
        xa = np.ascontiguousarray(a.transpose(1, 0, 2, 3))
        xb = np.ascontiguousarray(
            xl.reshape(NCC, 128, T).transpose(1, 0, 2))
        x12 = np.ascontiguousarray(xT[:VE_GATE_CH, :]).astype(BF)
        xbatch.append((xa, xb, x12))

    in_maps = []
    for core in range(N_CORES):
        b, g = divmod(core, KV)
        sl_q = slice(g * QD, (g + 1) * QD)
        sl_d = slice(g * D, (g + 1) * D)
        qa, qb = pack_w(Wq[sl_q])
        ka, kb = pack_w(Wk[sl_d])
        va, vb = pack_w(Wv[sl_d])
        xa, xb, x12 = xbatch[b]
        in_maps.append({
            "xA": xa, "xB": xb, "x12": x12,
            "wqA": qa, "wqB": qb, "wkA": ka, "wkB": kb,
            "wvA": va, "wvB": vb,
            "wpT": np.ascontiguousarray(
                Wproj[:, sl_q].T.reshape(REP, 128, C).transpose(1, 0, 2)
            ).astype(BF),
            "wg": np.ascontiguousarray(
                Wg[g].reshape(VE_GATE_CH, 1)).astype(BF),
            "veT": np.ascontiguousarray(
                (3.0 * WSC) * ve[b, :, sl_d].T).astype(BF),
            "cos2": cos2, "sin2m": sin2m, "eye": eye,
            "onesA": onesA, "masksI": masks,
        })
    return in_maps


def kernel(x, ve, cos, sin, Wq, Wk, Wv, Wproj, Wg, window, _trace=False):
    window = int(window)
    if window not in _CACHE:
        _CACHE[window] = _build(window)
    nc = _CACHE[window]

    in_maps = _prep_inputs(nc, window,
                           np.asarray(x, np.float32), np.asarray(ve, np.float32),
                           np.asarray(cos, np.float32), np.asarray(sin, np.float32),
                           np.asarray(Wq, np.float32), np.asarray(Wk, np.float32),
                           np.asarray(Wv, np.float32), np.asarray(Wproj, np.float32),
                           np.asarray(Wg, np.float32))

    res = run_bass_kernel_spmd(nc, in_maps, core_ids=list(range(N_CORES)),
                               trace=_trace)

    out = np.empty((B, T, C), dtype=np.float32)
    for b in range(B):
        acc = res.results[b * KV]["outT"].astype(np.float32)
        for g in range(1, KV):
            acc += res.results[b * KV + g]["outT"].astype(np.float32)
        out[b] = acc.T
    if _trace:
        kernel._last_trace = res
    return out
